# revision 11
# baseline (speedup 1.0000x reference)
"""Location-aware attention Bass/Trainium2 kernel.

Reference math (previous_attention=None, so location feats are 0):
    Wh      = hidden @ W_a_w.T + W_a_b                  [B, A]
    energy  = tanh(Wh[:, None, :] + 0)                  [B, S, A]
    scores  = energy @ V_a                              [B, S]
    scores  = where(mask, -inf, scores)
    weights = softmax(scores, axis=-1)                  [B, S]
    context = einsum('bs,bsh->bh', weights, enc)        [B, H]

Key identity: scores[b, s] = const[b] + maskterm[b, s] (the energy term is
independent of s because the location features are identically zero), and
softmax is shift-invariant per row, so weights == softmax(maskterm) exactly
for every row with at least one unmasked position. The kernel therefore
computes weights = softmax(-1e30 * mask) on device and spends its time on
the memory-bound context matvec over the 512 MiB encoder_outputs tensor.

Device layout notes:
  - The context matvec runs on the tensor engine: for each batch b and each
    128-row s-chunk k, matmul(psum[1, 512] += eT[:, k, b].T @ enc_chunk).
    eT (unnormalized exp of the mask term) is produced directly in the
    [s-on-partitions] layout from a host-transposed copy of the mask, so no
    on-device transposes are needed. The 1/sum normalization is folded into
    the psum->sbuf copy of the context row (exact for this problem: the
    weights are 1/4096, a power of two).
  - The [B, S] weights output is produced by an independent row-layout
    softmax off the critical path.
  - TRN2 PE matmuls can carry at most ONE semaphore wait (walrus codegen
    limit). Tiny "absorber" matmuls are inserted so that every heavy matmul
    has at most one cross-engine dependency left to wait on. build_nc()
    asserts this invariant post-build.

Sharding: data-parallel over batch, 4 batches per core on 8 cores.
"""

import numpy as np

B, S, H, A = 32, 4096, 1024, 512
NCORES = 8
BL = B // NCORES  # batches per core = 4
NEG = -1.0e30

_nc_cache = {}


def build_nc(s=S, ch=4, enc_bufs=8, reps=1):
    """Build the per-core Bass module. `s` is the sequence length (small for
    sim). `reps` repeats the whole computation back-to-back (for marginal
    exec-time measurement; outputs are identical each rep)."""
    import concourse.bacc as bacc
    import concourse.tile as tile
    from concourse import mybir

    f32 = mybir.dt.float32
    AF = mybir.ActivationFunctionType
    AX = mybir.AxisListType
    OP = mybir.AluOpType

    kc = s // 128          # number of 128-row s-chunks
    nt = kc // ch          # enc DMA tiles per batch
    assert kc % ch == 0

    # Bacc (not raw Bass): its compile() pass legalizes instructions that
    # carry more than one semaphore wait (walrus rejects those on TRN2).
    nc = bacc.Bacc("TRN2")

    enc = nc.dram_tensor("enc", [BL, s, H], f32, kind="ExternalInput")
    maskadd = nc.dram_tensor("maskadd", [BL, s], f32, kind="ExternalInput")
    maskadd_t = nc.dram_tensor("maskadd_t", [s, BL], f32, kind="ExternalInput")

    ctx_o = nc.dram_tensor("ctx", [BL, H], f32, kind="ExternalOutput")
    wts_o = nc.dram_tensor("wts", [BL, s], f32, kind="ExternalOutput")

    with tile.TileContext(nc) as tc:
        with (
            tc.tile_pool(name="consts", bufs=1 if reps == 1 else 2) as consts,
            tc.tile_pool(name="small", bufs=1 if reps == 1 else 2) as small,
            tc.tile_pool(name="rows", bufs=4) as rows,
            tc.tile_pool(name="encp", bufs=enc_bufs) as encp,
            tc.tile_pool(name="psum", bufs=2, space="PSUM") as psum,
        ):
          for _rep in range(reps):
            # ---------- critical path: eT = exp(maskT) in [s-part] layout ----------
            mT_sb = consts.tile([128, kc, BL], f32)
            nc.sync.dma_start(
                out=mT_sb, in_=maskadd_t.ap().rearrange("(k p) b -> p k b", p=128)
            )
            eT = consts.tile([128, kc, BL], f32)
            nc.scalar.activation(eT, mT_sb, AF.Exp)

            # absorber: a tiny matmul reading eT makes PE observe the ACT
            # tick, so every context matmul below waits only on its enc DMA
            dummy_ps = psum.tile([1, 1], f32, tag="dummy", bufs=1)
            nc.tensor.matmul(
                dummy_ps, lhsT=eT[:, 0, 0:1], rhs=eT[:, 0, 0:1],
                start=True, stop=True,
            )

            # ---------- off critical path: [B, S] weights output ----------
            mask_sb = consts.tile([BL, s], f32)
            nc.sync.dma_start(out=mask_sb, in_=maskadd.ap())
            e_row = small.tile([BL, s], f32)
            den_row = small.tile([BL, 1], f32)
            nc.scalar.activation(e_row, mask_sb, AF.Exp, accum_out=den_row)
            rec_row = small.tile([BL, 1], f32)
            nc.vector.reciprocal(rec_row, den_row)
            nc.vector.tensor_scalar_mul(e_row, in0=e_row, scalar1=rec_row)
            nc.sync.dma_start(out=wts_o.ap(), in_=e_row)

            # ---------- context[b, :] = rec[b] * sum_s eT[s, b] * enc[b, s, :] ----------
            # All 4 batches are packed into separate 32-column groups of the
            # PE array (tile_position col tiling), so the 4 matmuls of each
            # (chunk, half) run concurrently. psA/psB hold batch b's row at
            # partition 32*b.
            enc_r = enc.ap().rearrange("b (t p) h -> t p b h", p=128)
            psA = psum.tile([128, 512], f32, tag="psA", bufs=1)
            psB = psum.tile([128, 512], f32, tag="psB", bufs=1)
            for t in range(kc):
                et = encp.tile([128, BL, H], f32)
                nc.sync.dma_start(out=et, in_=enc_r[t])
                for half, ps in ((0, psA), (1, psB)):
                    for b in range(BL):
                        nc.tensor.matmul(
                            ps[32 * b : 32 * b + 1, :],
                            lhsT=eT[:, t, b : b + 1],
                            rhs=et[:, b, half * 512 : (half + 1) * 512],
                            start=(t == 0), stop=(t == kc - 1),
                            tile_position=(0, 32 * b),
                        )
            ctx_big = rows.tile([128, H], f32, tag="ctx_big", bufs=1)
            for b in range(BL):
                r = slice(32 * b, 32 * b + 1)
                nc.vector.tensor_copy(ctx_big[r, 0:512], psA[r, :])
                nc.vector.tensor_copy(ctx_big[r, 512:1024], psB[r, :])
            ctx4 = rows.tile([BL, H], f32, tag="ctx4", bufs=1)
            nc.sync.dma_start(
                out=ctx4,
                in_=ctx_big.rearrange("(a q) h -> a q h", q=32)[:, 0, :],
            )
            nc.vector.tensor_scalar_mul(ctx4, in0=ctx4, scalar1=rec_row)
            nc.sync.dma_start(out=ctx_o.ap(), in_=ctx4)

    nc.compile()
    return nc


def make_in_maps(hidden, encoder_outputs, mask, W_a_w, W_a_b, V_a_w):
    """Host-side layout marshalling + batch sharding. No math beyond dtype/layout."""
    encoder_outputs = np.asarray(encoder_outputs, dtype=np.float32)
    mask = np.asarray(mask)

    maskadd = np.where(mask, np.float32(NEG), np.float32(0.0))  # [B, S]

    in_maps = []
    for i in range(NCORES):
        bs = slice(i * BL, (i + 1) * BL)
        ma = np.ascontiguousarray(maskadd[bs])
        in_maps.append(
            {
                "enc": np.ascontiguousarray(encoder_outputs[bs]),
                "maskadd": ma,
                "maskadd_t": np.ascontiguousarray(ma.T),
            }
        )
    return in_maps


def kernel(hidden, encoder_outputs, mask, W_a_w, W_a_b, V_a_w):
    from concourse.bass_utils import run_bass_kernel_spmd

    if "nc" not in _nc_cache:
        _nc_cache["nc"] = build_nc()
    nc = _nc_cache["nc"]

    in_maps = make_in_maps(hidden, encoder_outputs, mask, W_a_w, W_a_b, V_a_w)
    res = run_bass_kernel_spmd(nc, in_maps, core_ids=list(range(NCORES)))
    context = np.concatenate([r["ctx"] for r in res.results], axis=0)
    weights = np.concatenate([r["wts"] for r in res.results], axis=0)
    return context, weights
